# revision 21
# baseline (speedup 1.0000x reference)
"""Trainium2 Bass kernel for Bert-AvgPooling-GCN (ragged sequence).

Strategy (8-core SPMD, one program, per-core data):
- Data-parallel over sentences: core c handles sentences [8c, 8c+8).
- All raggedness is encoded in per-core int32 index tensors consumed by
  indirect DMA, so a single compiled program serves all cores.
- Per-word token max: each word's tokens are CONSECUTIVE rows of
  sequence_output (token_indices = b*S + [0..ntok)), so the k-th token of
  word i lives at row start_i + min(k, t_i - 1). Four duplicate-capped row
  gathers + 3 elementwise maxes produce the padded word tensor
  x[s*120 + p] directly (pad rows are garbage; the zeroed adj columns make
  them irrelevant to valid outputs).
- GCN layer (x3): transpose h -> hT on PE; z = h @ W via float32r matmuls
  (full rate at N>=256); y = adjT_scaled @ z where adjT_scaled is
  transpose(adj / denom) computed once per sentence; h' = relu(y + b).
- Words scattered compactly (global word order) to DRAM scratch; clauses
  (5 consecutive global words) via 5 duplicate-capped gathers + max; FC to
  16 logits on device. Clauses split across a core boundary are exported
  as partial-max rows and combined on the host (<= 7 rows of 1180).
"""

import sys
from contextlib import ExitStack
import numpy as np

sys.path.insert(0, "/opt/trn_rl_repo")

B, S, D = 64, 512, 768
CLAUSE = 5
NCORES = 8
SPB = B // NCORES          # sentences per core
MW = 120                   # max words per sentence
WC = SPB * MW              # padded word slots per core (960)
CMAX = 160                 # padded clauses per core (two halves of 80)
CH = CMAX // 2
NEG = -1.0e30

MM_DT = "float32r"         # matmul dtype for the big GEMMs


def _build_program():
    import concourse.bass as bass
    import concourse.tile as tile
    from concourse import mybir
    from concourse.masks import make_identity

    f32 = mybir.dt.float32
    i32 = mybir.dt.int32
    mmdt = getattr(mybir.dt, MM_DT)
    Relu = mybir.ActivationFunctionType.Relu
    Copy = mybir.ActivationFunctionType.Copy

    nc = bass.Bass()

    seqc = nc.declare_dram_parameter("seqc", [SPB * S, D], f32, isOutput=False)
    adjc = nc.declare_dram_parameter("adjc", [SPB, MW, MW], f32, isOutput=False)
    Ws = [nc.declare_dram_parameter(f"W{l}", [D, D], f32, isOutput=False)
          for l in range(3)]
    bs = [nc.declare_dram_parameter(f"b{l}", [D], f32, isOutput=False)
          for l in range(3)]
    wfc = nc.declare_dram_parameter("wfc", [D, 16], f32, isOutput=False)
    bfc = nc.declare_dram_parameter("bfc", [16], f32, isOutput=False)
    gi = nc.declare_dram_parameter("gi", [SPB, MW], i32, isOutput=False)
    m3 = nc.declare_dram_parameter("m3", [SPB, MW, 3], f32, isOutput=False)
    cgab = nc.declare_dram_parameter("cgab", [2, 2, CH], i32, isOutput=False)
    cm10 = nc.declare_dram_parameter("cm10", [2, CH, 10], f32, isOutput=False)

    logits = nc.declare_dram_parameter("logits", [CMAX, 16], f32, isOutput=True)
    cmfull = nc.declare_dram_parameter("cmfull", [CMAX, D], f32, isOutput=True)

    hpad = nc.dram_tensor("hpad", [WC + 8, D], f32)

    KC = D // 128  # 6 contraction chunks

    with tile.TileContext(nc) as tc, ExitStack() as ctx:
        const = ctx.enter_context(tc.tile_pool(name="const", bufs=1))
        ipool = ctx.enter_context(tc.tile_pool(name="ipool", bufs=3))
        gpool = ctx.enter_context(tc.tile_pool(name="gpool", bufs=2))
        xpool = ctx.enter_context(tc.tile_pool(name="xpool", bufs=2))
        apool = ctx.enter_context(tc.tile_pool(name="apool", bufs=2))
        tpool = ctx.enter_context(tc.tile_pool(name="tpool", bufs=3))
        zpool = ctx.enter_context(tc.tile_pool(name="zpool", bufs=3))
        hpool = ctx.enter_context(tc.tile_pool(name="hpool", bufs=2))
        cpool = ctx.enter_context(tc.tile_pool(name="cpool", bufs=1))
        lpool = ctx.enter_context(tc.tile_pool(name="lpool", bufs=2))
        wstg_pool = ctx.enter_context(tc.tile_pool(name="wstg", bufs=1))
        tpsum = ctx.enter_context(tc.tile_pool(name="tpsum", bufs=2, space="PSUM"))
        zpsum = ctx.enter_context(tc.tile_pool(name="zpsum", bufs=2, space="PSUM"))
        ypsum = ctx.enter_context(tc.tile_pool(name="ypsum", bufs=2, space="PSUM"))

        ident = const.tile([128, 128], f32)
        make_identity(nc, ident[:])
        identr = const.tile([128, 128], mmdt)
        nc.vector.tensor_copy(out=identr[:], in_=ident[:])
        onesf = const.tile([1, MW], f32)
        nc.gpsimd.memset(onesf[:], 1.0)
        onesr = const.tile([1, MW], mmdt)
        nc.vector.tensor_copy(out=onesr[:], in_=onesf[:])

        # Weights as [128, KC, D] (partition = K % 128, chunk = K // 128)
        W_sb = []
        for l in range(3):
            wstg = wstg_pool.tile([128, KC, D], f32, tag="wstage")
            nc.sync.dma_start(out=wstg[:], in_=Ws[l].rearrange("(ko p) n -> p ko n", p=128))
            wt = const.tile([128, KC, D], mmdt, tag=f"W{l}")
            nc.vector.tensor_copy(out=wt[:], in_=wstg[:])
            W_sb.append(wt)
        wfc_sb = const.tile([128, KC, 16], f32)
        nc.sync.dma_start(out=wfc_sb[:], in_=wfc.rearrange("(ko p) n -> p ko n", p=128))

        brows = []
        for l in range(3):
            bstg = const.tile([1, D], f32, tag=f"bstg{l}")
            nc.scalar.dma_start(out=bstg[:], in_=bs[l][None, :])
            br = const.tile([1, D], mmdt, tag=f"brow{l}")
            nc.vector.tensor_copy(out=br[:], in_=bstg[:])
            brows.append(br)
        bfc_b = const.tile([128, 16], f32)
        nc.scalar.dma_start(out=bfc_b[:], in_=bfc[None, :].to_broadcast([128, 16]))

        gi_sb = const.tile([MW, SPB], i32)
        nc.sync.dma_start(out=gi_sb[:], in_=gi.rearrange("s p -> p s"))
        m3_sb = const.tile([MW, SPB, 3], f32)
        nc.sync.dma_start(out=m3_sb[:], in_=m3.rearrange("s p l -> p s l"))
        cg_sb = const.tile([CH, 2, 2], i32)
        nc.sync.dma_start(out=cg_sb[:], in_=cgab.rearrange("h a p -> p h a"))
        cm10_sb = const.tile([CH, 2, 10], f32)
        nc.sync.dma_start(out=cm10_sb[:], in_=cm10.rearrange("h p l -> p h l"))


        for s in range(SPB):
            gt = gpool.tile([MW, 4, D], f32, tag="g")
            nc.gpsimd.indirect_dma_start(
                out=gt[:].rearrange("p l d -> p (l d)"), out_offset=None, in_=seqc[:],
                in_offset=bass.IndirectOffsetOnAxis(ap=gi_sb[:, s:s + 1], axis=0))
            nc.vector.tensor_tensor(
                out=gt[:, 1:4, :], in0=gt[:, 1:4, :],
                in1=m3_sb[:, s, :, None].to_broadcast([MW, 3, D]),
                op=mybir.AluOpType.add)
            nc.vector.tensor_max(out=gt[:, 0, :], in0=gt[:, 0, :], in1=gt[:, 1, :])
            nc.vector.tensor_max(out=gt[:, 2, :], in0=gt[:, 2, :], in1=gt[:, 3, :])
            x = xpool.tile([MW, D], mmdt, tag="x")
            nc.vector.tensor_max(out=x[:], in0=gt[:, 0, :], in1=gt[:, 2, :])

            # adj prep: adjT_scaled = transpose(adj / (rowsum+1))
            adjld = apool.tile([MW, MW], f32, tag="adjld")
            nc.scalar.dma_start(out=adjld[:], in_=adjc[s])
            adj_t = adjld[:]
            dsum = apool.tile([MW, 1], f32, tag="dsum")
            nc.vector.tensor_reduce(out=dsum[:], in_=adj_t,
                                    axis=mybir.AxisListType.X, op=mybir.AluOpType.add)
            nc.vector.tensor_scalar_add(dsum[:], dsum[:], 1.0)
            rec = apool.tile([MW, 1], f32, tag="rec")
            nc.vector.reciprocal(rec[:], dsum[:])
            adj_sc = apool.tile([MW, MW], mmdt, tag="adjsc")
            nc.scalar.activation(out=adj_sc[:], in_=adj_t, func=Copy,
                                 scale=rec[:, :1])
            aT_full = tpsum.tile([128, KC, 128], mmdt, tag="tp", name="aT_full")
            nc.tensor.transpose(out=aT_full[:MW, 0, :MW], in_=adj_sc[:],
                                identity=identr[:MW, :MW])
            adjT = apool.tile([MW, MW], mmdt, tag="adjT")
            nc.vector.tensor_copy(out=adjT[:], in_=aT_full[:MW, 0, :MW])

            h = x
            for l in range(3):
                tp = tpsum.tile([128, KC, 128], mmdt, tag="tp")
                for c in range(KC):
                    nc.tensor.transpose(out=tp[:, c, :MW],
                                        in_=h[:, c * 128:(c + 1) * 128],
                                        identity=identr[:MW, :MW])
                hT = tpool.tile([128, KC, MW], mmdt, tag="hT")
                nc.vector.tensor_copy(out=hT[:], in_=tp[:, :, :MW])
                z_sb = zpool.tile([MW, 2, 384], mmdt, tag="z")
                hn = hpool.tile([MW, D], mmdt if l < 2 else f32, tag="h" + str(l % 2))
                for half in range(2):
                    nsl = slice(half * 384, (half + 1) * 384)
                    zp = zpsum.tile([MW, 384], f32, tag="zp")
                    for c in range(KC):
                        nc.tensor.matmul(
                            out=zp[:], lhsT=hT[:, c, :],
                            rhs=W_sb[l][:, c, nsl],
                            start=(c == 0), stop=(c == KC - 1))
                    nc.scalar.activation(out=z_sb[:, half, :], in_=zp[:], func=Copy)
                for half in range(2):
                    nsl = slice(half * 384, (half + 1) * 384)
                    yp = ypsum.tile([MW, 384], f32, tag="yp")
                    nc.tensor.matmul(out=yp[:], lhsT=adjT[:],
                                     rhs=z_sb[:, half, :],
                                     start=True, stop=False)
                    nc.tensor.matmul(out=yp[:], lhsT=onesr[:],
                                     rhs=brows[l][:, nsl],
                                     start=False, stop=True)
                    nc.scalar.activation(out=hn[:, nsl], in_=yp[:], func=Relu)
                h = hn

            nc.sync.dma_start(out=hpad[s * MW:(s + 1) * MW, :], in_=h[:])

        # clause stage: two halves of CH clauses; each clause is two 5-row
        # block gathers (run in first sentence + spill into next), lane-masked
        for half in range(2):
            ctA = cpool.tile([CH, 5, D], f32, tag="ctA")
            ctB = cpool.tile([CH, 5, D], f32, tag="ctB")
            nc.gpsimd.indirect_dma_start(
                out=ctA[:].rearrange("p l d -> p (l d)"), out_offset=None, in_=hpad[:],
                in_offset=bass.IndirectOffsetOnAxis(ap=cg_sb[:, half, 0:1], axis=0))
            nc.gpsimd.indirect_dma_start(
                out=ctB[:].rearrange("p l d -> p (l d)"), out_offset=None, in_=hpad[:],
                in_offset=bass.IndirectOffsetOnAxis(ap=cg_sb[:, half, 1:2], axis=0))
            nc.vector.tensor_tensor(
                out=ctA[:], in0=ctA[:],
                in1=cm10_sb[:, half, 0:5, None].to_broadcast([CH, 5, D]),
                op=mybir.AluOpType.add)
            nc.vector.tensor_tensor(
                out=ctB[:], in0=ctB[:],
                in1=cm10_sb[:, half, 5:10, None].to_broadcast([CH, 5, D]),
                op=mybir.AluOpType.add)
            nc.vector.tensor_max(out=ctA[:], in0=ctA[:], in1=ctB[:])
            cm = cpool.tile([CH, D], f32, tag="cm")
            nc.vector.tensor_max(out=cm[:], in0=ctA[:, 0, :], in1=ctA[:, 1, :])
            nc.vector.tensor_max(out=ctA[:, 2, :], in0=ctA[:, 2, :], in1=ctA[:, 3, :])
            nc.vector.tensor_max(out=cm[:], in0=cm[:], in1=ctA[:, 2, :])
            nc.vector.tensor_max(out=cm[:], in0=cm[:], in1=ctA[:, 4, :])
            nc.sync.dma_start(out=cmfull[half * CH:(half + 1) * CH, :], in_=cm[:])

            cT = tpool.tile([128, KC, CH], f32, tag="cT")
            for c in range(KC):
                ctpf = tpsum.tile([128, KC, 128], f32, tag="tp", name="ctpf")
                nc.tensor.transpose(out=ctpf[:, 0, :CH], in_=cm[:, c * 128:(c + 1) * 128],
                                    identity=ident[:CH, :CH])
                nc.vector.tensor_copy(out=cT[:, c, :], in_=ctpf[:, 0, :CH])
            lpf = ypsum.tile([MW, 384], f32, tag="yp", name="lpf")
            lp = lpf[:CH, :16]
            for c in range(KC):
                nc.tensor.matmul(out=lp, lhsT=cT[:, c, :], rhs=wfc_sb[:, c, :],
                                 start=(c == 0), stop=(c == KC - 1))
            lg = lpool.tile([CH, 16], f32, tag="lg")
            nc.vector.tensor_add(out=lg[:], in0=lp, in1=bfc_b[:CH, :])
            nc.sync.dma_start(out=logits[half * CH:(half + 1) * CH, :], in_=lg[:])

    _split_waits(nc, cap=1)
    return nc


def _split_waits(nc, cap=1):
    """Walrus in this toolchain rejects instructions carrying more than ~4
    semaphore waits. Split excess waits onto same-engine EventSemaphore nops
    inserted just before the instruction (engines process waits in program
    order, so this preserves the dependency semantics)."""
    from concourse import mybir
    ctr = 0
    for fn in nc.m.functions:
        for bb in fn.blocks:
            il = bb.instructions
            out = []
            changed = False
            for ins in il:
                si = ins.sync_info
                if si is not None and len(si.on_wait) > cap:
                    waits = list(si.on_wait)
                    head, tail = waits[:-cap], waits[-cap:]
                    for i in range(0, len(head), cap):
                        ctr += 1
                        out.append(mybir.InstEventSemaphore(
                            name=f"wsplit-{ctr}", engine=ins.engine, ins=[], outs=[],
                            sync_info=mybir.SyncInfo(on_wait=head[i:i + cap],
                                                     on_update=[])))
                    ins.sync_info = mybir.SyncInfo(on_wait=tail,
                                                   on_update=list(si.on_update))
                    changed = True
                out.append(ins)
            if changed:
                bb.instructions = out
    return nc


def _plan(token_indices, word_seg, word2sent, clause_seg, n_clauses):
    """Host-side per-core index/mask planning from the runtime ragged arrays."""
    token_indices = np.asarray(token_indices)
    word_seg = np.asarray(word_seg)
    word2sent = np.asarray(word2sent)

    Wt = word2sent.shape[0]
    w = np.bincount(word2sent, minlength=B)              # words per sentence
    sw = np.zeros(B + 1, np.int64)
    sw[1:] = np.cumsum(w)                                # sentence word starts
    t = np.bincount(word_seg, minlength=Wt)              # tokens per word
    wts = np.zeros(Wt + 1, np.int64)
    wts[1:] = np.cumsum(t)                               # word token starts
    row0 = token_indices[wts[:-1]]                       # first-token DRAM row

    cores = []
    for c in range(NCORES):
        b0 = c * SPB
        ws, we = int(sw[b0]), int(sw[b0 + SPB])
        wc = we - ws
        assert wc <= WC

        giv = np.zeros((SPB, MW), np.int32)
        m3v = np.full((SPB, MW, 3), NEG, np.float32)
        for ls in range(SPB):
            b = b0 + ls
            nw = int(w[b])
            gw = np.arange(sw[b], sw[b] + nw)
            giv[ls, :nw] = row0[gw] - b0 * S
            lane = np.arange(1, 4)[None, :]
            m3v[ls, :nw, :] = np.where(lane < t[gw][:, None], 0.0, NEG)
        assert giv.min() >= 0 and giv.max() + 4 <= SPB * S

        fc = ws // CLAUSE
        lc = (we - 1) // CLAUSE
        ncl = lc - fc + 1
        assert ncl <= CMAX and ncl - 1 >= CH, (c, ncl)

        cgabv = np.zeros((2, 2, CH), np.int32)
        cm10v = np.full((2, CH, 10), NEG, np.float32)
        for q in range(ncl):
            cid = fc + q
            hh, qq = divmod(q, CH)
            g0, g1 = CLAUSE * cid, CLAUSE * cid + 4     # global word range
            a0, a1 = max(g0, ws), min(g1, we - 1)       # present on this core
            bA = int(np.searchsorted(sw, a0, side="right") - 1)
            endA = min(a1, int(sw[bA + 1]) - 1)         # last present word in bA
            slotA = (bA - b0) * MW + (a0 - int(sw[bA]))
            cgabv[hh, 0, qq] = slotA
            nA = endA - a0 + 1
            cm10v[hh, qq, 0:nA] = 0.0
            if endA < a1:                               # spill into next sentence
                slotB = (bA + 1 - b0) * MW
                cgabv[hh, 1, qq] = slotB
                nB = a1 - endA
                cm10v[hh, qq, 5:5 + nB] = 0.0

        cores.append(dict(b0=b0, ws=ws, we=we, fc=fc, lc=lc, ncl=ncl,
                          gi=giv, m3=m3v, cgab=cgabv, cm10=cm10v))
    return cores


_CACHED = {}


def kernel(sequence_output, adj, W1, b1, W2, b2, W3, b3, Wfc, bfc,
           token_indices, word_seg, word2sent, word_pos, clause_seg, n_clauses):
    from concourse.bass_utils import run_bass_kernel_spmd

    sequence_output = np.asarray(sequence_output, dtype=np.float32)
    adj = np.asarray(adj, dtype=np.float32)
    weights = dict(W0=np.asarray(W1, np.float32), W1=np.asarray(W2, np.float32),
                   W2=np.asarray(W3, np.float32))
    biases = dict(b0=np.asarray(b1, np.float32), b1=np.asarray(b2, np.float32),
                  b2=np.asarray(b3, np.float32))
    wfc = np.asarray(Wfc, np.float32)
    bfcv = np.asarray(bfc, np.float32)
    n_clauses = int(n_clauses)

    cores = _plan(token_indices, word_seg, word2sent, clause_seg, n_clauses)

    if "nc" not in _CACHED:
        _CACHED["nc"] = _build_program()
    nc = _CACHED["nc"]

    in_maps = []
    for c, pl in enumerate(cores):
        b0 = pl["b0"]
        m = {
            "seqc": sequence_output[b0:b0 + SPB].reshape(SPB * S, D),
            "adjc": adj[b0:b0 + SPB],
            "wfc": wfc, "bfc": bfcv,
            "gi": pl["gi"], "m3": pl["m3"], "cgab": pl["cgab"], "cm10": pl["cm10"],
        }
        for l in range(3):
            m[f"W{l}"] = weights[f"W{l}"]
            m[f"b{l}"] = biases[f"b{l}"]
        in_maps.append(m)

    res = run_bass_kernel_spmd(nc, in_maps, core_ids=list(range(NCORES)))
    _CACHED["last_exec_time_ns"] = res.exec_time_ns

    out = np.zeros((n_clauses, 16), np.float32)
    for c, pl in enumerate(cores):
        out[pl["fc"]:pl["lc"] + 1] = res.results[c]["logits"][:pl["ncl"]]

    # clauses split across a core boundary: max-combine the two partial rows
    for c in range(NCORES - 1):
        we = cores[c]["we"]
        if we % CLAUSE != 0:
            cid = we // CLAUSE
            ncl_c = cores[c]["ncl"]
            a = res.results[c]["cmfull"][ncl_c - 1]
            bq = res.results[c + 1]["cmfull"][0]
            out[cid] = np.maximum(a, bq) @ wfc + bfcv
    return out
